# revision 1
# baseline (speedup 1.0000x reference)
"""Causal self-attention TRN2 kernel.

Full module: x[4,2048,1024] @ W_qkv[1024,3072] -> heads(16, d=64) causal attn
-> @ W_proj[1024,1024].

Sharding: 8 cores = 4 batches x 2 head-groups (8 heads each), tensor-parallel
over heads. Each core computes q/k/v for its 8 heads, causal attention, and a
partial projection (row-sharded W_proj). The two partials per batch are summed
on the host (no on-device collectives).

Per-core program, pipelined over 4 token blocks of 512 (block qc produces the
QKV slices that attention q-chunk qc consumes, so exp/attention work overlaps
later blocks' projection matmuls):
  QKV:   qT quarters [128f, 512t] (head-pair-major features), kT chunks
         [128f, T], V tiles [128t, 8 heads, 64 V + 1 ones col] (bf16);
         x is fed pre-transposed and streamed in 512-token quarters so
         compute starts ~2 MiB into the input stream. Matmuls in f32r
         (full PE rate at N=512, ~FP22 precision).
  attn:  scores^T [k,q] per 128-k-tile: the head pair is row-packed into one
         PE pass (K=64 halves at tile_position (0,0)/(64,0)) writing one
         2-bank PSUM tile; ONE exp per k-tile on ACT covers both heads
         ([128,1024], scale=1/8 folded into the activation); causal masking
         multiplies only the 128-col triangular block (+memset for fully
         masked cols) of diagonal tiles; y^T += [V|1]^T @ P~ accumulates in
         PSUM with row 64 = softmax denominators; divide via DVE
         reciprocal + K=1 matmul partition-broadcast of 1/denom. All matmul
         operands are f32r: self-loading matmuls only (a separate LDWEIGHTS
         can be silicon-reordered ahead of its data-ready wait and read a
         freshly-written lhsT tile before its producer lands).
  proj:  out[t,:] partial = yT^T @ wp per q-chunk, f32r.

build_nc(repeat=R) emits the whole computation R times (optionally
dependency-serialized) for wall-clock-differenced device timing.
"""

import numpy as np
from contextlib import ExitStack

import concourse.bass as bass
import concourse.tile as tile
from concourse import mybir, bacc
from concourse.bass_utils import run_bass_kernel_spmd

F32 = mybir.dt.float32
F32R = mybir.dt.float32r
BF16 = mybir.dt.bfloat16
EXP = mybir.ActivationFunctionType.Exp

B, T, C, H, D = 4, 2048, 1024, 16, 64
NCORES = 8
GROUPS = 2            # head groups (tensor-parallel dimension)
HPC = H // GROUPS     # heads per core = 8
FPC = HPC * D         # features per core = 512
SCALE = 1.0 / np.sqrt(D)


def build_nc(T=T, C=C, HPC=HPC, repeat=1, serialize_reps=False):
    FPC = HPC * D
    NC = C // 128     # contraction chunks over C
    NT = T // 128     # token tiles (also k-tiles)
    NQ = T // 512     # query chunks (= merged pipeline blocks)
    NF = FPC // 128   # feature tiles = head pairs
    NN = max(C // 512, 1)  # proj output column chunks
    npj = min(512, C)

    nc = bacc.Bacc("TRN2", debug=False)
    xT_d = nc.dram_tensor("xT", [C, T], F32R, kind="ExternalInput").ap()
    wq_d = nc.dram_tensor("wq", [C, FPC], F32R, kind="ExternalInput").ap()
    wk_d = nc.dram_tensor("wk", [C, FPC], F32R, kind="ExternalInput").ap()
    wv_d = nc.dram_tensor("wv", [C, FPC], F32R, kind="ExternalInput").ap()
    wp_d = nc.dram_tensor("wp", [FPC, C], F32R, kind="ExternalInput").ap()
    mk_d = nc.dram_tensor("trimask", [128, 128], F32R, kind="ExternalInput").ap()
    on_d = nc.dram_tensor("ones64", [1, 64], F32R, kind="ExternalInput").ap()
    ov_d = nc.dram_tensor("onesv", [128, 8, 1], F32R, kind="ExternalInput").ap()
    zr_d = nc.dram_tensor("zeros", [128, 384], F32R, kind="ExternalInput").ap()
    out_d = nc.dram_tensor("out", [T, C], F32, kind="ExternalOutput").ap()

    with tile.TileContext(nc) as tc, ExitStack() as ctx:
        p_kt = ctx.enter_context(tc.tile_pool(name="p_kt", bufs=NF))
        p_v65 = ctx.enter_context(tc.tile_pool(name="p_v65", bufs=NT))
        p_const = ctx.enter_context(tc.tile_pool(name="p_const", bufs=1))
        p_w = ctx.enter_context(tc.tile_pool(name="p_w", bufs=NF * NC))
        p_wv = ctx.enter_context(tc.tile_pool(name="p_wv", bufs=NC))
        p_xq = ctx.enter_context(tc.tile_pool(name="p_xq", bufs=8))
        p_qtq = ctx.enter_context(tc.tile_pool(name="p_qtq", bufs=2 * NF))
        p_ytq = ctx.enter_context(tc.tile_pool(name="p_ytq", bufs=2 * NF))
        p_pt = ctx.enter_context(tc.tile_pool(name="p_pt", bufs=3))
        p_rec = ctx.enter_context(tc.tile_pool(name="p_rec", bufs=1))
        p_ys = ctx.enter_context(tc.tile_pool(name="p_ys", bufs=2))
        p_ybt = ctx.enter_context(tc.tile_pool(name="p_ybt", bufs=1))
        p_wp = ctx.enter_context(tc.tile_pool(name="p_wp", bufs=NF))
        p_osb = ctx.enter_context(tc.tile_pool(name="p_osb", bufs=2))
        # one shared PSUM budget, 8 banks: s 2x2 + y 2 + misc 2
        ps_s = ctx.enter_context(tc.tile_pool(name="ps_s", bufs=2, space="PSUM"))
        ps_y = ctx.enter_context(tc.tile_pool(name="ps_y", bufs=2, space="PSUM"))
        ps_m = ctx.enter_context(tc.tile_pool(name="ps_m", bufs=2, space="PSUM"))

        # constants
        ones_t = p_const.tile([65, 64], F32R, tag="ones")
        nc.sync.dma_start(out=ones_t[64:65, :], in_=on_d[:])
        trimask = p_const.tile([128, 128], F32R, tag="trimask")
        nc.sync.dma_start(out=trimask[:], in_=mk_d[:])

        kt_ = [p_kt.tile([128, T], F32R, tag="kt", name=f"kt{i}") for i in range(NF)]
        v65 = [p_v65.tile([128, HPC, 65], F32R, tag="v65", name=f"v65_{i}")
               for i in range(NT)]

        wq_t, wk_t = {}, {}

        serdep = {"on": False}

        def dma_w(n, f):
            for c in range(NC):
                for w_d, store, wtag in ((wq_d, wq_t, "wq"), (wk_d, wk_t, "wk")):
                    wt = p_w.tile([128, 128], F32R, tag=wtag,
                                  name=f"{wtag}{n}_{f}_{c}")
                    src_ap = w_d[c * 128:(c + 1) * 128, f * 128:(f + 1) * 128]
                    if serdep["on"] and n == 0 and f == 0:
                        # timing mode: source the first weight tile from the
                        # previous repeat's final output rows to serialize reps
                        src_ap = out_d[(NT - 1) * 128:NT * 128, 0:128].bitcast(F32R)
                    nc.sync.dma_start(out=wt[:], in_=src_ap)
                    store[(f, c)] = wt

        xq = {}

        def dma_xq(n):
            for c in range(NC):
                t_ = p_xq.tile([128, 512], F32R, tag="xq", name=f"xq{c}_{n}")
                nc.sync.dma_start(
                    out=t_[:], in_=xT_d[c * 128:(c + 1) * 128,
                                        n * 512:(n + 1) * 512])
                xq[(c, n)] = t_

        def emit_once():
            qtq = {}   # (f, qc) -> [128, 512] query quarter
            ytq = {}   # (f, qc) -> [128, 512] attention-out quarter
            wq_t.clear()
            wk_t.clear()
            xq.clear()
            dma_w(0, 0)
            dma_xq(0)
            for f in range(1, NF):
                dma_w(0, f)
            wv_sb = []
            for c in range(NC):
                t_ = p_wv.tile([128, FPC], F32R, tag="wv")
                nc.sync.dma_start(out=t_[:], in_=wv_d[c * 128:(c + 1) * 128, :])
                wv_sb.append(t_)
            wp_sb = []
            for cf in range(NF):
                wt = p_wp.tile([128, C], F32R, tag="wp", name=f"wp{cf}")
                nc.sync.dma_start(out=wt[:], in_=wp_d[cf * 128:(cf + 1) * 128, :])
                wp_sb.append(wt)

            def q_group(n, f, isq):
                store = wq_t if isq else wk_t
                ps = ps_m.tile([128, 512], F32, tag="m1", name=f"qk{n}_{f}")
                for c in range(NC):
                    nc.tensor.matmul(
                        ps[:], store[(f, c)][:], xq[(c, n)][:],
                        start=(c == 0), stop=(c == NC - 1))
                if isq:
                    dst = p_qtq.tile([128, 512], F32R, tag="qt",
                                     name=f"qtq{f}_{n}")
                    qtq[(f, n)] = dst
                    nc.vector.tensor_copy(out=dst[:], in_=ps[:])
                else:
                    nc.vector.tensor_copy(
                        out=kt_[f][:, n * 512:(n + 1) * 512], in_=ps[:])

            def v_group(n, t):
                ps = ps_m.tile([128, FPC], F32, tag="m1", name=f"v{t}")
                for c in range(NC):
                    nc.tensor.matmul(
                        ps[:], xq[(c, n)][:, (t % 4) * 128:(t % 4 + 1) * 128],
                        wv_sb[c][:], start=(c == 0), stop=(c == NC - 1))
                nc.vector.tensor_copy(
                    out=v65[t][:, :, 0:64],
                    in_=ps[:].rearrange("p (h d) -> p h d", h=HPC))
                nc.sync.dma_start(out=v65[t][:, :, 64:65], in_=ov_d[:, 0:HPC, :])

            def qkv_groups(n):
                gs = []
                for f in range(NF):
                    def g(f=f):
                        if n > 0:
                            dma_w(n, f)
                        q_group(n, f, True)
                    gs.append(g)
                for f in range(NF):
                    gs.append(lambda f=f: q_group(n, f, False))
                for t in range(4 * n, 4 * n + 4):
                    gs.append(lambda t=t: v_group(n, t))
                return gs

            def attention_hp(qc, hp):
                nk = 4 * qc + 4
                qsl = slice(qc * 512, (qc + 1) * 512)
                y_psA = ps_y.tile([65, 512], F32, tag="y")
                y_psB = ps_y.tile([65, 512], F32, tag="y")
                qtile = qtq[(hp, qc)]
                for kt in range(nk):
                    s_ps = ps_s.tile([128, 1024], F32, tag="s")
                    nc.tensor.matmul(
                        s_ps[:, 0:512],
                        kt_[hp][0:64, kt * 128:(kt + 1) * 128],
                        qtile[0:64, :],
                        start=True, stop=True, tile_position=(0, 0))
                    nc.tensor.matmul(
                        s_ps[:, 512:1024],
                        kt_[hp][64:128, kt * 128:(kt + 1) * 128],
                        qtile[64:128, :],
                        start=True, stop=True, tile_position=(64, 0))
                    pt = p_pt.tile([128, 1024], F32R, tag="pt")
                    nc.scalar.activation(
                        out=pt[:], in_=s_ps[:], func=EXP, scale=float(SCALE))
                    if kt >= 4 * qc:
                        d = kt - 4 * qc
                        dcol = 128 * d
                        nc.vector.tensor_mul(
                            pt[:, dcol:dcol + 128], pt[:, dcol:dcol + 128],
                            trimask[:])
                        nc.vector.tensor_mul(
                            pt[:, 512 + dcol:512 + dcol + 128],
                            pt[:, 512 + dcol:512 + dcol + 128], trimask[:])
                        if d > 0:
                            nc.sync.dma_start(out=pt[:, 0:dcol],
                                              in_=zr_d[:, 0:dcol])
                            nc.sync.dma_start(out=pt[:, 512:512 + dcol],
                                              in_=zr_d[:, 0:dcol])
                    nc.tensor.matmul(
                        y_psA[:], v65[kt][:, 2 * hp, :], pt[:, 0:512],
                        start=(kt == 0), stop=(kt == nk - 1))
                    nc.tensor.matmul(
                        y_psB[:], v65[kt][:, 2 * hp + 1, :], pt[:, 512:1024],
                        start=(kt == 0), stop=(kt == nk - 1))

                # softmax division; stage y psum to SBUF immediately so the
                # accumulator banks free for the next block
                ysA = p_ys.tile([65, 512], F32, tag="ys")
                nc.vector.tensor_copy(out=ysA[:], in_=y_psA[:])
                ysB = p_ys.tile([65, 512], F32, tag="ys")
                nc.vector.tensor_copy(out=ysB[:], in_=y_psB[:])

                ytile = p_ytq.tile([128, 512], F32R, tag="yt",
                                   name=f"ytq{hp}_{qc}")
                ytq[(hp, qc)] = ytile

                recA = p_rec.tile([65, 512], F32R, tag="rec")
                with nc.allow_low_precision("f32r softmax denom reciprocal"):
                    nc.vector.reciprocal(out=recA[64:65, :], in_=ysA[64:65, :])
                bcA = ps_m.tile([64, 512], F32, tag="m1")
                nc.tensor.matmul(
                    bcA[:], ones_t[64:65, :], recA[64:65, :],
                    start=True, stop=True, tile_position=(64, 0))
                nc.vector.tensor_mul(ytile[0:64, :], ysA[0:64, :], bcA[:])

                recB = p_rec.tile([65, 512], F32R, tag="rec")
                with nc.allow_low_precision("f32r softmax denom reciprocal"):
                    nc.vector.reciprocal(out=recB[64:65, :], in_=ysB[64:65, :])
                bcB = ps_m.tile([64, 512], F32, tag="m1")
                nc.tensor.matmul(
                    bcB[:], ones_t[64:65, :], recB[64:65, :],
                    start=True, stop=True, tile_position=(64, 0))
                ybt = p_ybt.tile([64, 512], F32R, tag="ybt")
                nc.vector.tensor_mul(ybt[:], ysB[0:64, :], bcB[:])
                nc.sync.dma_start(out=ytile[64:128, :], in_=ybt[:])

            def proj_t(qc, t):
                tloc = (t - 4 * qc) * 128
                osb = p_osb.tile([128, C], F32, tag="osb", name=f"osb{t}")
                for nn in range(NN):
                    pj = ps_m.tile([128, npj], F32, tag="m1", name=f"pj{t}_{nn}")
                    for cf in range(NF):
                        nc.tensor.matmul(
                            pj[:],
                            ytq[(cf, qc)][:, tloc:tloc + 128],
                            wp_sb[cf][:, nn * npj:(nn + 1) * npj],
                            start=(cf == 0), stop=(cf == NF - 1))
                    nc.vector.tensor_copy(
                        out=osb[:, nn * npj:(nn + 1) * npj], in_=pj[:])
                nc.sync.dma_start(
                    out=out_d[t * 128:(t + 1) * 128, :], in_=osb[:])

            # sequential per-block emission: QKV block qc, then attention
            # for q-chunk qc, then its projection (finer-grained interleaved
            # emission was measured racy on HW; the Tile scheduler still
            # overlaps phases via dataflow dependencies)
            for qc in range(NQ):
                if qc + 1 < NQ:
                    dma_xq(qc + 1)
                for g in qkv_groups(qc):
                    g()
                for hp in range(NF):
                    attention_hp(qc, hp)
                for t in range(4 * qc, 4 * qc + 4):
                    proj_t(qc, t)

        for _rep in range(repeat):
            serdep["on"] = serialize_reps and _rep > 0
            emit_once()
    nc.finalize()
    return nc


def _make_masks():
    kk = np.arange(128)[:, None]
    jj = np.arange(128)[None, :]
    return (jj >= kk).astype(np.float32)


def make_in_maps(x, W_qkv, W_proj):
    """Host-side sharding of full inputs into per-core input maps."""
    x = np.asarray(x, dtype=np.float32)
    W_qkv = np.asarray(W_qkv, dtype=np.float32)
    W_proj = np.asarray(W_proj, dtype=np.float32)
    masks = _make_masks()
    in_maps = []
    for core in range(NCORES):
        b, g = core // GROUPS, core % GROUPS
        in_maps.append({
            "xT": np.ascontiguousarray(x[b].T),
            "wq": np.ascontiguousarray(W_qkv[:, g * FPC:(g + 1) * FPC]),
            "wk": np.ascontiguousarray(W_qkv[:, C + g * FPC:C + (g + 1) * FPC]),
            "wv": np.ascontiguousarray(W_qkv[:, 2 * C + g * FPC:2 * C + (g + 1) * FPC]),
            "wp": np.ascontiguousarray(W_proj[g * FPC:(g + 1) * FPC, :]),
            "trimask": masks,
            "ones64": np.ones((1, 64), np.float32),
            "onesv": _ones_bf16(),
            "zeros": np.zeros((128, 384), np.float32),
        })
    return in_maps


def _ones_bf16():
    return np.ones((128, 8, 1), dtype=np.float32)


_CACHE = {}


def _get_nc():
    if "nc" not in _CACHE:
        _CACHE["nc"] = build_nc()
    return _CACHE["nc"]


def run_cores(in_maps):
    res = run_bass_kernel_spmd(_get_nc(), in_maps, list(range(NCORES)))
    return res.results


def kernel(x, W_qkv, W_proj):
    results = run_cores(make_in_maps(x, W_qkv, W_proj))
    out = np.empty((B, T, C), dtype=np.float32)
    for b in range(B):
        out[b] = results[GROUPS * b]["out"]
        for g in range(1, GROUPS):
            out[b] += results[GROUPS * b + g]["out"]
    return out



# revision 4
# speedup vs baseline: 1051.6470x; 1051.6470x over previous
"""Causal self-attention TRN2 kernel (v2).

Full module: x[4,2048,1024] @ W_qkv[1024,3072] -> heads(16, d=64) causal attn
-> @ W_proj[1024,1024].

Sharding: 8 cores = 4 batches x 2 head-groups (8 heads each), tensor-parallel
over heads. Each core computes q/k/v for its 8 heads, causal attention, and a
partial projection (row-sharded W_proj). The two partials per batch are summed
on the host (no on-device collectives).

v2 changes vs the f32r baseline:
  - All streamed tensors fp16 (x, W slices, K/V/Q tiles, P~, y, output);
    PSUM accumulation stays fp32.  Halves HBM+host transfer bytes, enables
    DVE 2x packed modes and PE fast-weight-load.
  - x uploaded in natural [T, C] layout; the kernel DMA-transposes 512x128
    blocks into xT tiles on the way into SBUF (xbar transpose) instead of a
    host-side np transpose.
  - Causal masking: the fully-masked column range of diagonal score tiles is
    never exp'd (live-range ACT + DVE memset) instead of exp-then-zero-DMA
    from an HBM zeros tensor; the triangular block of both heads is masked by
    ONE strided tensor_mul against a duplicated [128,2,128] mask.
  - Weight loads batched to [128, 512] row-chunks (4x fewer DMAs); V-ones
    column + broadcast-ones row via memset, not DMA.
  - build_nc(loop_n=R) wraps the whole per-pass computation in a tc.For_i
    hardware loop (same NEFF size for any R) for wall-clock-differenced
    device timing.

Per-core program, per 512-token q-chunk qc (as baseline):
  QKV:   qT quarters [128f, 512t] (head-pair-major features), kT chunks
         [128f, T], V tiles [128t, 8 heads, 64 V + 1 ones col]. Matmuls fp16
         in, fp32 PSUM out.
  attn:  scores^T [k,q] per 128-k-tile: head pair row-packed into one PE pass
         (K=64 halves at tile_position (0,0)/(64,0)) writing one 2-bank PSUM
         tile; ONE exp per k-tile on ACT covers both heads (scale=1/8 folded
         in); y^T += [V|1]^T @ P~ accumulates in PSUM with row 64 = softmax
         denominators; divide via DVE reciprocal + K=1 matmul
         partition-broadcast of 1/denom.
  proj:  out[t,:] partial = yT^T @ wp per q-chunk.
"""

import numpy as np
import ml_dtypes
from contextlib import ExitStack

import concourse.bass as bass
import concourse.tile as tile
from concourse import mybir, bacc
from concourse.bass_utils import run_bass_kernel_spmd

F32 = mybir.dt.float32
F16 = mybir.dt.float16
EXP = mybir.ActivationFunctionType.Exp

B, T, C, H, D = 4, 2048, 1024, 16, 64
NCORES = 8
GROUPS = 2            # head groups (tensor-parallel dimension)
HPC = H // GROUPS     # heads per core = 8
FPC = HPC * D         # features per core = 512
SCALE = 1.0 / np.sqrt(D)

NPF16 = ml_dtypes.bfloat16  # placeholder overwritten below
NPF16 = np.float16


def build_nc(T=T, C=C, HPC=HPC, loop_n=None):
    FPC = HPC * D
    NC = C // 128     # contraction chunks over C
    NT = T // 128     # token tiles (also k-tiles)
    NQ = T // 512     # query chunks (= merged pipeline blocks)
    NF = FPC // 128   # feature tiles = head pairs
    NN = max(C // 512, 1)  # proj output column chunks
    npj = min(512, C)

    nc = bacc.Bacc("TRN2", debug=False)
    x_d = nc.dram_tensor("x", [T, C], F16, kind="ExternalInput").ap()
    wq_d = nc.dram_tensor("wq", [C, FPC], F16, kind="ExternalInput").ap()
    wk_d = nc.dram_tensor("wk", [C, FPC], F16, kind="ExternalInput").ap()
    wv_d = nc.dram_tensor("wv", [C, FPC], F16, kind="ExternalInput").ap()
    wp_d = nc.dram_tensor("wp", [FPC, C], F16, kind="ExternalInput").ap()
    mk_d = nc.dram_tensor("trimask2", [128, 2, 128], F16, kind="ExternalInput").ap()
    out_d = nc.dram_tensor("out", [T, C], F16, kind="ExternalOutput").ap()

    with tile.TileContext(nc) as tc, ExitStack() as ctx:
        p_kt = ctx.enter_context(tc.tile_pool(name="p_kt", bufs=NF))
        p_v65 = ctx.enter_context(tc.tile_pool(name="p_v65", bufs=NT))
        p_const = ctx.enter_context(tc.tile_pool(name="p_const", bufs=1))
        p_w = ctx.enter_context(tc.tile_pool(name="p_w", bufs=2 * NC))
        p_wv = ctx.enter_context(tc.tile_pool(name="p_wv", bufs=NC))
        p_xq = ctx.enter_context(tc.tile_pool(name="p_xq", bufs=NC * NQ))
        p_qtq = ctx.enter_context(tc.tile_pool(name="p_qtq", bufs=2 * NF))
        p_ytq = ctx.enter_context(tc.tile_pool(name="p_ytq", bufs=2 * NF))
        p_pt = ctx.enter_context(tc.tile_pool(name="p_pt", bufs=3))
        p_rec = ctx.enter_context(tc.tile_pool(name="p_rec", bufs=1))
        p_ys = ctx.enter_context(tc.tile_pool(name="p_ys", bufs=2))
        p_ybt = ctx.enter_context(tc.tile_pool(name="p_ybt", bufs=1))
        p_wp = ctx.enter_context(tc.tile_pool(name="p_wp", bufs=NF))
        p_osb = ctx.enter_context(tc.tile_pool(name="p_osb", bufs=2))
        # one shared PSUM budget, 8 banks: s 2x2 + y 2 + misc 2
        ps_s = ctx.enter_context(tc.tile_pool(name="ps_s", bufs=2, space="PSUM"))
        ps_y = ctx.enter_context(tc.tile_pool(name="ps_y", bufs=2, space="PSUM"))
        ps_m = ctx.enter_context(tc.tile_pool(name="ps_m", bufs=2, space="PSUM"))

        kt_ = [p_kt.tile([128, T], F16, tag="kt", name=f"kt{i}") for i in range(NF)]
        v65 = [p_v65.tile([128, HPC, 65], F16, tag="v65", name=f"v65_{i}")
               for i in range(NT)]

        def emit_once():
            # constants (tiny; re-done per pass so the hw loop stays honest)
            ones_t = p_const.tile([65, 64], F16, tag="ones")
            nc.vector.memset(ones_t[64:65, :], 1.0)
            trimask = p_const.tile([128, 2, 128], F16, tag="trimask")
            nc.sync.dma_start(out=trimask[:], in_=mk_d[:])

            qtq = {}   # (f, qc) -> [128, 512] query quarter
            ytq = {}   # (f, qc) -> [128, 512] attention-out quarter
            xq = {}    # (c, n)  -> [128, 512] xT tile (DMA-transposed)

            # all x tiles up-front via xbar transpose (fp16, HBM->SBUF)
            for n in range(NQ):
                for c in range(NC):
                    t_ = p_xq.tile([128, 512], F16, tag="xq", name=f"xq{c}_{n}")
                    nc.sync.dma_start(
                        out=t_[:],
                        in_=x_d[n * 512:(n + 1) * 512, c * 128:(c + 1) * 128],
                        transpose=True)
                    xq[(c, n)] = t_

            # weights, batched by 128-row chunk
            wq_sb, wk_sb, wv_sb = [], [], []
            for c in range(NC):
                for w_d, store, wtag in ((wq_d, wq_sb, "wq"), (wk_d, wk_sb, "wk")):
                    wt = p_w.tile([128, FPC], F16, tag=wtag, name=f"{wtag}{c}")
                    nc.sync.dma_start(out=wt[:], in_=w_d[c * 128:(c + 1) * 128, :])
                    store.append(wt)
                t_ = p_wv.tile([128, FPC], F16, tag="wv", name=f"wv{c}")
                nc.sync.dma_start(out=t_[:], in_=wv_d[c * 128:(c + 1) * 128, :])
                wv_sb.append(t_)
            wp_sb = []
            for cf in range(NF):
                wt = p_wp.tile([128, C], F16, tag="wp", name=f"wp{cf}")
                nc.sync.dma_start(out=wt[:], in_=wp_d[cf * 128:(cf + 1) * 128, :])
                wp_sb.append(wt)

            def q_group(n, f, isq):
                store = wq_sb if isq else wk_sb
                ps = ps_m.tile([128, 512], F32, tag="m1", name=f"qk{n}_{f}")
                for c in range(NC):
                    nc.tensor.matmul(
                        ps[:], store[c][:, f * 128:(f + 1) * 128], xq[(c, n)][:],
                        start=(c == 0), stop=(c == NC - 1))
                if isq:
                    dst = p_qtq.tile([128, 512], F16, tag="qt",
                                     name=f"qtq{f}_{n}")
                    qtq[(f, n)] = dst
                    nc.vector.tensor_copy(out=dst[:], in_=ps[:])
                else:
                    nc.vector.tensor_copy(
                        out=kt_[f][:, n * 512:(n + 1) * 512], in_=ps[:])

            def v_group(n, t):
                ps = ps_m.tile([128, FPC], F32, tag="m1", name=f"v{t}")
                for c in range(NC):
                    nc.tensor.matmul(
                        ps[:], xq[(c, n)][:, (t % 4) * 128:(t % 4 + 1) * 128],
                        wv_sb[c][:], start=(c == 0), stop=(c == NC - 1))
                nc.vector.tensor_copy(
                    out=v65[t][:, :, 0:64],
                    in_=ps[:].rearrange("p (h d) -> p h d", h=HPC))
                nc.gpsimd.memset(v65[t][:, :, 64:65], 1.0)

            def qkv_groups(n):
                gs = []
                for f in range(NF):
                    gs.append(lambda f=f: q_group(n, f, True))
                for f in range(NF):
                    gs.append(lambda f=f: q_group(n, f, False))
                for t in range(4 * n, 4 * n + 4):
                    gs.append(lambda t=t: v_group(n, t))
                return gs

            def attention_hp(qc, hp):
                nk = 4 * qc + 4
                y_psA = ps_y.tile([65, 512], F32, tag="y")
                y_psB = ps_y.tile([65, 512], F32, tag="y")
                qtile = qtq[(hp, qc)]
                for kt in range(nk):
                    s_ps = ps_s.tile([128, 2, 512], F32, tag="s")
                    nc.tensor.matmul(
                        s_ps[:, 0, :],
                        kt_[hp][0:64, kt * 128:(kt + 1) * 128],
                        qtile[0:64, :],
                        start=True, stop=True, tile_position=(0, 0))
                    nc.tensor.matmul(
                        s_ps[:, 1, :],
                        kt_[hp][64:128, kt * 128:(kt + 1) * 128],
                        qtile[64:128, :],
                        start=True, stop=True, tile_position=(64, 0))
                    pt = p_pt.tile([128, 2, 512], F16, tag="pt")
                    d = kt - 4 * qc
                    if kt < 4 * qc or d == 0:
                        # fully live, or diagonal with no dead prefix
                        nc.scalar.activation(
                            out=pt[:], in_=s_ps[:], func=EXP, scale=float(SCALE))
                    else:
                        dcol = 128 * d
                        nc.scalar.activation(
                            out=pt[:, :, dcol:512], in_=s_ps[:, :, dcol:512],
                            func=EXP, scale=float(SCALE))
                        nc.gpsimd.memset(pt[:, :, 0:dcol], 0.0)
                    if d >= 0:
                        dcol = 128 * d
                        nc.vector.tensor_mul(
                            pt[:, :, dcol:dcol + 128], pt[:, :, dcol:dcol + 128],
                            trimask[:])
                    nc.tensor.matmul(
                        y_psA[:], v65[kt][:, 2 * hp, :], pt[:, 0, :],
                        start=(kt == 0), stop=(kt == nk - 1))
                    nc.tensor.matmul(
                        y_psB[:], v65[kt][:, 2 * hp + 1, :], pt[:, 1, :],
                        start=(kt == 0), stop=(kt == nk - 1))

                # softmax division; stage y psum to SBUF immediately so the
                # accumulator banks free for the next block
                ysA = p_ys.tile([65, 512], F32, tag="ys")
                nc.vector.tensor_copy(out=ysA[:], in_=y_psA[:])
                ysB = p_ys.tile([65, 512], F32, tag="ys")
                nc.vector.tensor_copy(out=ysB[:], in_=y_psB[:])

                ytile = p_ytq.tile([128, 512], F16, tag="yt",
                                   name=f"ytq{hp}_{qc}")
                ytq[(hp, qc)] = ytile

                recA = p_rec.tile([65, 512], F16, tag="rec")
                with nc.allow_low_precision("fp16 softmax denom reciprocal"):
                    nc.vector.reciprocal(out=recA[64:65, :], in_=ysA[64:65, :])
                bcA = ps_m.tile([64, 512], F32, tag="m1")
                nc.tensor.matmul(
                    bcA[:], ones_t[64:65, :], recA[64:65, :],
                    start=True, stop=True, tile_position=(64, 0))
                nc.vector.tensor_mul(ytile[0:64, :], ysA[0:64, :], bcA[:])

                recB = p_rec.tile([65, 512], F16, tag="rec")
                with nc.allow_low_precision("fp16 softmax denom reciprocal"):
                    nc.vector.reciprocal(out=recB[64:65, :], in_=ysB[64:65, :])
                bcB = ps_m.tile([64, 512], F32, tag="m1")
                nc.tensor.matmul(
                    bcB[:], ones_t[64:65, :], recB[64:65, :],
                    start=True, stop=True, tile_position=(64, 0))
                ybt = p_ybt.tile([64, 512], F16, tag="ybt")
                nc.vector.tensor_mul(ybt[:], ysB[0:64, :], bcB[:])
                nc.sync.dma_start(out=ytile[64:128, :], in_=ybt[:])

            def proj_t(qc, t):
                tloc = (t - 4 * qc) * 128
                osb = p_osb.tile([128, C], F16, tag="osb", name=f"osb{t}")
                for nn in range(NN):
                    pj = ps_m.tile([128, npj], F32, tag="m1", name=f"pj{t}_{nn}")
                    for cf in range(NF):
                        nc.tensor.matmul(
                            pj[:],
                            ytq[(cf, qc)][:, tloc:tloc + 128],
                            wp_sb[cf][:, nn * npj:(nn + 1) * npj],
                            start=(cf == 0), stop=(cf == NF - 1))
                    nc.vector.tensor_copy(
                        out=osb[:, nn * npj:(nn + 1) * npj], in_=pj[:])
                nc.sync.dma_start(
                    out=out_d[t * 128:(t + 1) * 128, :], in_=osb[:])

            # sequential per-block emission: QKV block qc, then attention
            # for q-chunk qc, then its projection (the Tile scheduler still
            # overlaps phases via dataflow dependencies)
            for qc in range(NQ):
                for g in qkv_groups(qc):
                    g()
                for hp in range(NF):
                    attention_hp(qc, hp)
                for t in range(4 * qc, 4 * qc + 4):
                    proj_t(qc, t)

        if loop_n is None:
            emit_once()
        else:
            with tc.For_i(0, int(loop_n), 1):
                emit_once()
    nc.finalize()
    return nc


def _make_masks():
    kk = np.arange(128)[:, None]
    jj = np.arange(128)[None, :]
    m = (jj >= kk).astype(NPF16)          # [k, q] lower-left live (q >= k)
    return np.ascontiguousarray(np.broadcast_to(m[:, None, :], (128, 2, 128)))


def make_in_maps(x, W_qkv, W_proj):
    """Host-side sharding of full inputs into per-core input maps (fp16)."""
    x = np.asarray(x)
    W_qkv = np.asarray(W_qkv)
    W_proj = np.asarray(W_proj)
    xh = [np.ascontiguousarray(x[b], dtype=NPF16) for b in range(B)]
    masks = _make_masks()
    in_maps = []
    for core in range(NCORES):
        b, g = core // GROUPS, core % GROUPS
        in_maps.append({
            "x": xh[b],
            "wq": np.ascontiguousarray(
                W_qkv[:, g * FPC:(g + 1) * FPC], dtype=NPF16),
            "wk": np.ascontiguousarray(
                W_qkv[:, C + g * FPC:C + (g + 1) * FPC], dtype=NPF16),
            "wv": np.ascontiguousarray(
                W_qkv[:, 2 * C + g * FPC:2 * C + (g + 1) * FPC], dtype=NPF16),
            "wp": np.ascontiguousarray(
                W_proj[g * FPC:(g + 1) * FPC, :], dtype=NPF16),
            "trimask2": masks,
        })
    return in_maps


_CACHE = {}


def _get_nc():
    if "nc" not in _CACHE:
        _CACHE["nc"] = build_nc()
    return _CACHE["nc"]


def run_cores(in_maps):
    res = run_bass_kernel_spmd(_get_nc(), in_maps, list(range(NCORES)))
    return res.results


def kernel(x, W_qkv, W_proj):
    results = run_cores(make_in_maps(x, W_qkv, W_proj))
    out = np.empty((B, T, C), dtype=np.float32)
    for b in range(B):
        out[b] = results[GROUPS * b]["out"].astype(np.float32)
        for g in range(1, GROUPS):
            out[b] += results[GROUPS * b + g]["out"].astype(np.float32)
    return out


# revision 45
# speedup vs baseline: 1315.3765x; 1.2508x over previous
"""Causal self-attention TRN2 kernel (v2).

Full module: x[4,2048,1024] @ W_qkv[1024,3072] -> heads(16, d=64) causal attn
-> @ W_proj[1024,1024].

Sharding: 8 cores = 4 batches x 2 head-groups (8 heads each), tensor-parallel
over heads. Each core computes q/k/v for its 8 heads, causal attention, and a
partial projection (row-sharded W_proj). The two partials per batch are summed
on the host (no on-device collectives).

v2 changes vs the f32r baseline (517 -> ~400 us/pass measured via the
hardware-loop differencing in test.py):
  - All streamed tensors fp16 (xT, packed W_qkv, W_proj, K/V/Q tiles, P~, y,
    output); PSUM accumulation stays fp32.  Halves HBM bytes, enables DVE 2x
    packed modes and PE fast-weight-load.  Measured end-to-end rel err ~5e-4
    (tolerance 2e-2).
  - DMA traffic spread over all three parallel issue paths: xT tiles on the
    two HWDGE rings (nc.sync + nc.scalar), weights / output / small copies
    on the GpSimd SWDGE queues; weights coalesced to ONE strided DMA each.
  - Causal masking: the fully-masked column range of diagonal score tiles is
    never exp'd (live-range ACT + DVE memset) instead of exp-then-zero-DMA
    from an HBM zeros tensor; the triangular block of both heads is masked
    by ONE strided tensor_mul against a duplicated [128,2,128] mask.
  - 1/denom via ACT ln->exp(-x) instead of DVE reciprocal (iterative divide,
    ~8 cyc/elem = ~4.3us per row); both heads' denom rows in one ln and one
    exp.  A scoped patch keeps Exp+Ln in the single
    natural_log_exp_and_others activation-table set (no 2.7us set thrash).
  - Software-pipelined emission: QKV of chunk qc+1 before attention of qc,
    proj of qc one chunk late, so the Tile scheduler always has independent
    PE work to fill attention's cross-engine dependency stalls.
  - build_nc(loop_n=R) wraps the whole per-pass computation in a tc.For_i
    hardware loop (same NEFF size for any R) for wall-clock-differenced
    device timing; staggered=True variant exists but measured slower.

Per-core program, per 512-token q-chunk qc (as baseline):
  QKV:   qT quarters [128f, 512t] (head-pair-major features), kT chunks
         [128f, T], V tiles [128t, 8 heads, 64 V + 1 ones col]. Matmuls fp16
         in, fp32 PSUM out.
  attn:  scores^T [k,q] per 128-k-tile: head pair row-packed into one PE pass
         (K=64 halves at tile_position (0,0)/(64,0), concurrent on HW)
         writing one 2-bank PSUM tile; ONE exp per k-tile on ACT covers both
         heads (scale=1/8 folded in); y^T += [V|1]^T @ P~ accumulates in
         PSUM with row 64 = softmax denominators; divide via ln/exp + K=1
         matmul partition-broadcast of 1/denom.
  proj:  out[t,:] partial = yT^T @ wp per q-chunk, stored fp16.
"""

import numpy as np
import ml_dtypes
from contextlib import ExitStack

import concourse.bass as bass
import concourse.tile as tile
from concourse import mybir, bacc
from concourse.bass_utils import run_bass_kernel_spmd

F32 = mybir.dt.float32
F16 = mybir.dt.float16
EXP = mybir.ActivationFunctionType.Exp
LOG = mybir.ActivationFunctionType.Ln

B, T, C, H, D = 4, 2048, 1024, 16, 64
NCORES = 8
GROUPS = 2            # head groups (tensor-parallel dimension)
HPC = H // GROUPS     # heads per core = 8
FPC = HPC * D         # features per core = 512
SCALE = 1.0 / np.sqrt(D)

NPF16 = ml_dtypes.bfloat16  # placeholder overwritten below
NPF16 = np.float16


def build_nc(T=T, C=C, HPC=HPC, loop_n=None, parts="full", tune=None,
             ablate=None, staggered=False):
    tu = {"pt": 5, "ys": 2, "rec": 1, "osb": 2, "ybt": 1, "qtq": None,
          "ytq": 12}
    if tune:
        tu.update(tune)
    FPC = HPC * D
    NC = C // 128     # contraction chunks over C
    NT = T // 128     # token tiles (also k-tiles)
    NQ = T // 512     # query chunks (= merged pipeline blocks)
    NF = FPC // 128   # feature tiles = head pairs
    NN = max(C // 512, 1)  # proj output column chunks
    npj = min(512, C)

    nc = bacc.Bacc("TRN2", debug=False)
    x_d = nc.dram_tensor("xT", [C, T], F16, kind="ExternalInput").ap()
    wqkv_d = nc.dram_tensor("wqkv", [C, 3 * FPC], F16, kind="ExternalInput").ap()
    wp_d = nc.dram_tensor("wp", [FPC, C], F16, kind="ExternalInput").ap()
    mk_d = nc.dram_tensor("trimask2", [128, 2, 128], F16, kind="ExternalInput").ap()
    out_d = nc.dram_tensor("out", [T, C], F16, kind="ExternalOutput").ap()

    with tile.TileContext(nc) as tc, ExitStack() as ctx:
        p_kt = ctx.enter_context(tc.tile_pool(name="p_kt", bufs=NF))
        p_v65 = ctx.enter_context(tc.tile_pool(name="p_v65", bufs=NT))
        p_const = ctx.enter_context(tc.tile_pool(name="p_const", bufs=1))
        p_w = ctx.enter_context(tc.tile_pool(name="p_w", bufs=1))
        p_xq = ctx.enter_context(tc.tile_pool(name="p_xq", bufs=NC))
        p_qtq = ctx.enter_context(
            tc.tile_pool(name="p_qtq", bufs=tu["qtq"] or 2 * NF))
        p_ytq = ctx.enter_context(
            tc.tile_pool(name="p_ytq", bufs=tu["ytq"] or 2 * NF))
        p_pt = ctx.enter_context(tc.tile_pool(name="p_pt", bufs=tu["pt"]))
        p_rec = ctx.enter_context(tc.tile_pool(name="p_rec", bufs=tu["rec"]))
        p_ys = ctx.enter_context(tc.tile_pool(name="p_ys", bufs=tu["ys"]))
        p_ybt = ctx.enter_context(tc.tile_pool(name="p_ybt", bufs=tu["ybt"]))
        p_wp = ctx.enter_context(tc.tile_pool(name="p_wp", bufs=1))
        p_osb = ctx.enter_context(tc.tile_pool(name="p_osb", bufs=tu["osb"]))
        # one shared PSUM budget, 8 banks: s 2x2 + y 2 + misc 2
        ps_s = ctx.enter_context(tc.tile_pool(name="ps_s", bufs=2, space="PSUM"))
        ps_y = ctx.enter_context(tc.tile_pool(name="ps_y", bufs=2, space="PSUM"))
        ps_m = ctx.enter_context(tc.tile_pool(name="ps_m", bufs=2, space="PSUM"))

        kt_ = [p_kt.tile([128, T], F16, tag="kt", name=f"kt{i}") for i in range(NF)]
        v65 = [p_v65.tile([128, HPC, 65], F16, tag="v65", name=f"v65_{i}")
               for i in range(NT)]

        def emit_once(stage_cb=None):
            # x^T tiles (host pre-transposed [C, T] fp16), one [128, T] tile
            # per 128-feature chunk.  Plain DMA; in staggered mode these are
            # stage 0 (they overlap the previous iteration's tail) and must
            # all issue from SP, which is idle there; otherwise alternate
            # between the two HWDGE rings (SP via nc.sync, ACT via
            # nc.scalar).
            xqc = []
            for c in range(NC):
                t_ = p_xq.tile([128, T], F16, tag="xq", name=f"xq{c}")
                eng = nc.sync if (stage_cb or c % 2 == 0) else nc.scalar
                eng.dma_start(out=t_[:],
                              in_=x_d[c * 128:(c + 1) * 128, :])
                xqc.append(t_)
            if stage_cb:
                stage_cb()

            # constants (tiny; re-done per pass so the hw loop stays honest)
            ones_t = p_const.tile([65, 64], F16, tag="ones")
            nc.vector.memset(ones_t[64:65, :], 1.0)
            trimask = p_const.tile([128, 2, 128], F16, tag="trimask")
            nc.gpsimd.dma_start(out=trimask[:], in_=mk_d[:])

            qtq = {}   # (f, qc) -> [128, 512] query quarter
            ytq = {}   # (f, qc) -> [128, 512] attention-out quarter

            def xq(c, n):
                return xqc[c][:, n * 512:(n + 1) * 512]

            # weights: one strided SWDGE DMA each (GpSimd queues — off the
            # HWDGE rings the transposes are using)
            w_sb = p_w.tile([128, NC, 3 * FPC], F16, tag="wqkv")
            nc.gpsimd.dma_start(
                out=w_sb[:], in_=wqkv_d.rearrange("(c p) f -> p c f", p=128))
            wp_sb = p_wp.tile([128, NF, C], F16, tag="wp")
            nc.gpsimd.dma_start(
                out=wp_sb[:], in_=wp_d.rearrange("(cf p) j -> p cf j", p=128))

            def q_group(n, f, isq):
                off = 0 if isq else FPC
                ps = ps_m.tile([128, 512], F32, tag="m1", name=f"qk{n}_{f}")
                for c in range(NC):
                    nc.tensor.matmul(
                        ps[:], w_sb[:, c, off + f * 128:off + (f + 1) * 128],
                        xq(c, n),
                        start=(c == 0), stop=(c == NC - 1))
                if isq:
                    dst = p_qtq.tile([128, 512], F16, tag="qt",
                                     name=f"qtq{f}_{n}")
                    qtq[(f, n)] = dst
                    nc.vector.tensor_copy(out=dst[:], in_=ps[:])
                else:
                    nc.vector.tensor_copy(
                        out=kt_[f][:, n * 512:(n + 1) * 512], in_=ps[:])

            def v_group(n, t):
                ps = ps_m.tile([128, FPC], F32, tag="m1", name=f"v{t}")
                for c in range(NC):
                    nc.tensor.matmul(
                        ps[:], xqc[c][:, t * 128:(t + 1) * 128],
                        w_sb[:, c, 2 * FPC:3 * FPC],
                        start=(c == 0), stop=(c == NC - 1))
                nc.vector.tensor_copy(
                    out=v65[t][:, :, 0:64],
                    in_=ps[:].rearrange("p (h d) -> p h d", h=HPC))
                nc.gpsimd.memset(v65[t][:, :, 64:65], 1.0)

            def qkv_groups(n):
                gs = []
                for f in range(NF):
                    gs.append(lambda f=f: q_group(n, f, True))
                for f in range(NF):
                    gs.append(lambda f=f: q_group(n, f, False))
                for t in range(4 * n, 4 * n + 4):
                    gs.append(lambda t=t: v_group(n, t))
                return gs

            pt0 = None
            if ablate == "noexp":
                pt0 = p_const.tile([128, 2, 512], F16, tag="pt0")
                nc.vector.memset(pt0[:], 0.002)

            def attention_hp(qc, hp):
                nk = 4 * qc + 4
                y_psA = ps_y.tile([65, 512], F32, tag="y")
                y_psB = ps_y.tile([65, 512], F32, tag="y")
                qtile = qtq[(hp, qc)]
                for kt in range(nk):
                    s_ps = ps_s.tile([128, 2, 512], F32, tag="s")
                    nc.tensor.matmul(
                        s_ps[:, 0, :],
                        kt_[hp][0:64, kt * 128:(kt + 1) * 128],
                        qtile[0:64, :],
                        start=True, stop=True, tile_position=(0, 0))
                    nc.tensor.matmul(
                        s_ps[:, 1, :],
                        kt_[hp][64:128, kt * 128:(kt + 1) * 128],
                        qtile[64:128, :],
                        start=True, stop=True, tile_position=(64, 0))
                    if ablate == "noexp":
                        nc.tensor.matmul(
                            y_psA[:], v65[kt][:, 2 * hp, :], pt0[:, 0, :],
                            start=(kt == 0), stop=(kt == nk - 1))
                        nc.tensor.matmul(
                            y_psB[:], v65[kt][:, 2 * hp + 1, :], pt0[:, 1, :],
                            start=(kt == 0), stop=(kt == nk - 1))
                        continue
                    pt = p_pt.tile([128, 2, 512], F16, tag="pt")
                    d = kt - 4 * qc
                    if kt < 4 * qc or d == 0:
                        # fully live, or diagonal with no dead prefix
                        nc.scalar.activation(
                            out=pt[:], in_=s_ps[:], func=EXP, scale=float(SCALE))
                    else:
                        dcol = 128 * d
                        nc.scalar.activation(
                            out=pt[:, :, dcol:512], in_=s_ps[:, :, dcol:512],
                            func=EXP, scale=float(SCALE))
                        nc.vector.memset(pt[:, :, 0:dcol], 0.0)
                    if d >= 0:
                        dcol = 128 * d
                        nc.vector.tensor_mul(
                            pt[:, :, dcol:dcol + 128], pt[:, :, dcol:dcol + 128],
                            trimask[:])
                    nc.tensor.matmul(
                        y_psA[:], v65[kt][:, 2 * hp, :], pt[:, 0, :],
                        start=(kt == 0), stop=(kt == nk - 1))
                    nc.tensor.matmul(
                        y_psB[:], v65[kt][:, 2 * hp + 1, :], pt[:, 1, :],
                        start=(kt == 0), stop=(kt == nk - 1))

                # softmax division; stage y psum to SBUF immediately so the
                # accumulator banks free for the next block
                ys = p_ys.tile([65, 2, 512], F32, tag="ys")
                nc.vector.tensor_copy(out=ys[:, 0, :], in_=y_psA[:])
                nc.vector.tensor_copy(out=ys[:, 1, :], in_=y_psB[:])

                ytile = p_ytq.tile([128, 512], F16, tag="yt",
                                   name=f"ytq{hp}_{qc}")
                ytq[(hp, qc)] = ytile

                # 1/denom via ACT ln -> exp(-x): DVE reciprocal is an
                # iterative-divide (~8 cyc/elem, ~4.3us per row); ln+exp are
                # two ACT ops covering BOTH heads' denom rows, sharing one
                # activation table set with the attention exps
                # (natural_log_exp).
                ln = p_rec.tile([65, 2, 512], F32, tag="ln")
                nc.scalar.activation(out=ln[64:65, :, :], in_=ys[64:65, :, :],
                                     func=LOG)
                rec = p_rec.tile([65, 2, 512], F16, tag="rec")
                nc.scalar.activation(out=rec[64:65, :, :], in_=ln[64:65, :, :],
                                     func=EXP, scale=-1.0)
                bcA = ps_m.tile([64, 512], F32, tag="m1")
                nc.tensor.matmul(
                    bcA[:], ones_t[64:65, :], rec[64:65, 0, :],
                    start=True, stop=True, tile_position=(64, 0))
                nc.vector.tensor_mul(ytile[0:64, :], ys[0:64, 0, :], bcA[:])

                bcB = ps_m.tile([64, 512], F32, tag="m1")
                nc.tensor.matmul(
                    bcB[:], ones_t[64:65, :], rec[64:65, 1, :],
                    start=True, stop=True, tile_position=(64, 0))
                ybt = p_ybt.tile([64, 512], F16, tag="ybt")
                nc.vector.tensor_mul(ybt[:], ys[0:64, 1, :], bcB[:])
                nc.gpsimd.dma_start(out=ytile[64:128, :], in_=ybt[:])

            def proj_t(qc, t, osb):
                tloc = (t - 4 * qc) * 128
                for nn in range(NN):
                    pj = ps_m.tile([128, npj], F32, tag="m1", name=f"pj{t}_{nn}")
                    for cf in range(NF):
                        nc.tensor.matmul(
                            pj[:],
                            ytq[(cf, qc)][:, tloc:tloc + 128],
                            wp_sb[:, cf, nn * npj:(nn + 1) * npj],
                            start=(cf == 0), stop=(cf == NF - 1))
                    nc.vector.tensor_copy(
                        out=osb[:, t - 4 * qc, nn * npj:(nn + 1) * npj], in_=pj[:])

            def proj_block(qc):
                osb = p_osb.tile([128, 4, C], F16, tag="osb", name=f"osb{qc}")
                for th in range(2):
                    for t in range(4 * qc + 2 * th, 4 * qc + 2 * th + 2):
                        proj_t(qc, t, osb)
                    nc.gpsimd.dma_start(
                        out=out_d[qc * 512 + th * 256:
                                  qc * 512 + (th + 1) * 256, :].rearrange(
                            "(tt p) j -> p tt j", p=128),
                        in_=osb[:, 2 * th:2 * th + 2, :])

            # software-pipelined emission: QKV for chunk qc+1 is emitted
            # BEFORE attention of chunk qc, and proj for chunk qc is emitted
            # one chunk late, so the scheduler always has independent PE work
            # (qkv early, proj late) to fill attention's dependency stalls —
            # in particular the last chunk's attention, which has no qkv
            # filler left.  Tile still tracks all dataflow deps.
            for g in qkv_groups(0):
                g()
            for qc in range(NQ):
                if qc + 1 < NQ:
                    for g in qkv_groups(qc + 1):
                        g()
                if parts == "qkv":
                    continue
                for hp in range(NF):
                    attention_hp(qc, hp)
                if parts == "attn":
                    continue
                if qc - 1 >= 0:
                    proj_block(qc - 1)
                if stage_cb and qc in (0, 2):
                    stage_cb()
            if parts == "full":
                proj_block(NQ - 1)
            if parts == "qkv":
                # timing-only variant: consume q/k/v so nothing is dead
                for f in range(NF):
                    nc.sync.dma_start(out=out_d[f * 128:(f + 1) * 128, 0:512],
                                      in_=qtq[(f, NQ - 1)][:])
            elif parts == "attn":
                for f in range(NF):
                    nc.sync.dma_start(out=out_d[f * 128:(f + 1) * 128, 0:512],
                                      in_=ytq[(f, NQ - 1)][:])

        if loop_n is None:
            emit_once()
        elif staggered:
            assert parts == "full" and ablate is None
            with tc.For_i(0, int(loop_n), 1, staggered_reset=True):
                emit_once(stage_cb=tc.stage_boundary)
        else:
            with tc.For_i(0, int(loop_n), 1):
                emit_once()

    # The greedy act-table-load pass picks a table set per activation; with
    # Exp resolving to "exp_and_others" and Ln to
    # "natural_log_exp_and_others" it would thrash sets (~2.7us per reload,
    # 65 reloads).  Keep the original set order (act_func_set_id is an index
    # into act_info.json) but hide Exp from every other set, so both Exp and
    # Ln resolve to the one set containing both -> exactly one load.
    import concourse.bacc as _bacc_mod
    _orig_tables = _bacc_mod.get_activation_tables

    def _tables_ln_exp_only(arch):
        tabs = _orig_tables(arch)
        both = "natural_log_exp_and_others"
        if both in tabs:
            for name, fns in tabs.items():
                if name != both:
                    fns.discard(EXP)
        return tabs

    _bacc_mod.get_activation_tables = _tables_ln_exp_only
    try:
        nc.finalize()
    finally:
        _bacc_mod.get_activation_tables = _orig_tables
    return nc


def _make_masks():
    kk = np.arange(128)[:, None]
    jj = np.arange(128)[None, :]
    m = (jj >= kk).astype(NPF16)          # [k, q] lower-left live (q >= k)
    return np.ascontiguousarray(np.broadcast_to(m[:, None, :], (128, 2, 128)))


def make_in_maps(x, W_qkv, W_proj):
    """Host-side sharding of full inputs into per-core input maps (fp16)."""
    x = np.asarray(x)
    W_qkv = np.asarray(W_qkv)
    W_proj = np.asarray(W_proj)
    xh = [np.ascontiguousarray(x[b].T, dtype=NPF16) for b in range(B)]
    masks = _make_masks()
    wqkv = [np.concatenate(
        [W_qkv[:, s * C + g * FPC:s * C + (g + 1) * FPC] for s in range(3)],
        axis=1).astype(NPF16) for g in range(GROUPS)]
    wps = [np.ascontiguousarray(W_proj[g * FPC:(g + 1) * FPC, :], dtype=NPF16)
           for g in range(GROUPS)]
    in_maps = []
    for core in range(NCORES):
        b, g = core // GROUPS, core % GROUPS
        in_maps.append({
            "xT": xh[b],
            "wqkv": wqkv[g],
            "wp": wps[g],
            "trimask2": masks,
        })
    return in_maps


_CACHE = {}


def _get_nc():
    if "nc" not in _CACHE:
        _CACHE["nc"] = build_nc()
    return _CACHE["nc"]


def run_cores(in_maps):
    res = run_bass_kernel_spmd(_get_nc(), in_maps, list(range(NCORES)))
    return res.results


def kernel(x, W_qkv, W_proj):
    results = run_cores(make_in_maps(x, W_qkv, W_proj))
    out = np.empty((B, T, C), dtype=np.float32)
    for b in range(B):
        out[b] = results[GROUPS * b]["out"].astype(np.float32)
        for g in range(1, GROUPS):
            out[b] += results[GROUPS * b + g]["out"].astype(np.float32)
    return out


# revision 49
# speedup vs baseline: 1344.2922x; 1.0220x over previous
"""Causal self-attention TRN2 kernel (v2).

Full module: x[4,2048,1024] @ W_qkv[1024,3072] -> heads(16, d=64) causal attn
-> @ W_proj[1024,1024].

Sharding: 8 cores = 4 batches x 2 head-groups (8 heads each), tensor-parallel
over heads. Each core computes q/k/v for its 8 heads, causal attention, and a
partial projection (row-sharded W_proj). The two partials per batch are summed
on the host (no on-device collectives).

v2 changes vs the f32r baseline (517 -> ~400 us/pass measured via the
hardware-loop differencing in test.py):
  - All streamed tensors fp16 (xT, packed W_qkv, W_proj, K/V/Q tiles, P~, y,
    output); PSUM accumulation stays fp32.  Halves HBM bytes, enables DVE 2x
    packed modes and PE fast-weight-load.  Measured end-to-end rel err ~5e-4
    (tolerance 2e-2).
  - DMA traffic spread over all three parallel issue paths: xT tiles on the
    two HWDGE rings (nc.sync + nc.scalar), weights / output / small copies
    on the GpSimd SWDGE queues; weights coalesced to ONE strided DMA each.
  - Causal masking: the fully-masked column range of diagonal score tiles is
    never exp'd (live-range ACT + DVE memset) instead of exp-then-zero-DMA
    from an HBM zeros tensor; the triangular block of both heads is masked
    by ONE strided tensor_mul against a duplicated [128,2,128] mask.
  - 1/denom via ACT ln->exp(-x) instead of DVE reciprocal (iterative divide,
    ~8 cyc/elem = ~4.3us per row); both heads' denom rows in one ln and one
    exp.  A scoped patch keeps Exp+Ln in the single
    natural_log_exp_and_others activation-table set (no 2.7us set thrash).
  - Software-pipelined emission: QKV of chunk qc+1 before attention of qc,
    proj of qc one chunk late, so the Tile scheduler always has independent
    PE work to fill attention's cross-engine dependency stalls.
  - build_nc(loop_n=R) wraps the whole per-pass computation in a tc.For_i
    hardware loop (same NEFF size for any R) for wall-clock-differenced
    device timing; staggered=True variant exists but measured slower.

Per-core program, per 512-token q-chunk qc (as baseline):
  QKV:   qT quarters [128f, 512t] (head-pair-major features), kT chunks
         [128f, T], V tiles [128t, 8 heads, 64 V + 1 ones col]. Matmuls fp16
         in, fp32 PSUM out.
  attn:  scores^T [k,q] per 128-k-tile: head pair row-packed into one PE pass
         (K=64 halves at tile_position (0,0)/(64,0), concurrent on HW)
         writing one 2-bank PSUM tile; ONE exp per k-tile on ACT covers both
         heads (scale=1/8 folded in); y^T += [V|1]^T @ P~ accumulates in
         PSUM with row 64 = softmax denominators; divide via ln/exp + K=1
         matmul partition-broadcast of 1/denom.
  proj:  out[t,:] partial = yT^T @ wp per q-chunk, stored fp16.
"""

import numpy as np
from contextlib import ExitStack

import concourse.bass as bass
import concourse.tile as tile
from concourse import mybir, bacc
from concourse.bass_utils import run_bass_kernel_spmd

F32 = mybir.dt.float32
F16 = mybir.dt.float16
EXP = mybir.ActivationFunctionType.Exp
LOG = mybir.ActivationFunctionType.Ln

B, T, C, H, D = 4, 2048, 1024, 16, 64
NCORES = 8
GROUPS = 2            # head groups (tensor-parallel dimension)
HPC = H // GROUPS     # heads per core = 8
FPC = HPC * D         # features per core = 512
SCALE = 1.0 / np.sqrt(D)

NPF16 = np.float16  # host-side dtype matching the F16 device tensors


def build_nc(T=T, C=C, HPC=HPC, loop_n=None, parts="full", tune=None,
             ablate=None, staggered=False):
    tu = {"pt": 5, "ys": 2, "rec": 1, "osb": 2, "ybt": 1, "qtq": None,
          "ytq": 12}
    if tune:
        tu.update(tune)
    FPC = HPC * D
    NC = C // 128     # contraction chunks over C
    NT = T // 128     # token tiles (also k-tiles)
    NQ = T // 512     # query chunks (= merged pipeline blocks)
    NF = FPC // 128   # feature tiles = head pairs
    NN = max(C // 512, 1)  # proj output column chunks
    npj = min(512, C)

    nc = bacc.Bacc("TRN2", debug=False)
    x_d = nc.dram_tensor("xT", [C, T], F16, kind="ExternalInput").ap()
    wqkv_d = nc.dram_tensor("wqkv", [C, 3 * FPC], F16, kind="ExternalInput").ap()
    wp_d = nc.dram_tensor("wp", [FPC, C], F16, kind="ExternalInput").ap()
    mk_d = nc.dram_tensor("trimask2", [128, 2, 128], F16, kind="ExternalInput").ap()
    out_d = nc.dram_tensor("out", [T, C], F16, kind="ExternalOutput").ap()

    with tile.TileContext(nc) as tc, ExitStack() as ctx:
        p_kt = ctx.enter_context(tc.tile_pool(name="p_kt", bufs=NF))
        p_v65 = ctx.enter_context(tc.tile_pool(name="p_v65", bufs=NT))
        p_const = ctx.enter_context(tc.tile_pool(name="p_const", bufs=1))
        p_w = ctx.enter_context(tc.tile_pool(name="p_w", bufs=1))
        p_xq = ctx.enter_context(tc.tile_pool(name="p_xq", bufs=NC))
        p_qtq = ctx.enter_context(
            tc.tile_pool(name="p_qtq", bufs=tu["qtq"] or 2 * NF))
        p_ytq = ctx.enter_context(
            tc.tile_pool(name="p_ytq", bufs=tu["ytq"] or 2 * NF))
        p_pt = ctx.enter_context(tc.tile_pool(name="p_pt", bufs=tu["pt"]))
        p_rec = ctx.enter_context(tc.tile_pool(name="p_rec", bufs=tu["rec"]))
        p_ys = ctx.enter_context(tc.tile_pool(name="p_ys", bufs=tu["ys"]))
        p_ybt = ctx.enter_context(tc.tile_pool(name="p_ybt", bufs=tu["ybt"]))
        p_wp = ctx.enter_context(tc.tile_pool(name="p_wp", bufs=1))
        p_osb = ctx.enter_context(tc.tile_pool(name="p_osb", bufs=tu["osb"]))
        # one shared PSUM budget, 8 banks: s 2x2 + y 2 + misc 2
        ps_s = ctx.enter_context(tc.tile_pool(name="ps_s", bufs=2, space="PSUM"))
        ps_y = ctx.enter_context(tc.tile_pool(name="ps_y", bufs=2, space="PSUM"))
        ps_m = ctx.enter_context(tc.tile_pool(name="ps_m", bufs=2, space="PSUM"))

        kt_ = [p_kt.tile([128, T], F16, tag="kt", name=f"kt{i}") for i in range(NF)]
        v65 = [p_v65.tile([128, HPC, 65], F16, tag="v65", name=f"v65_{i}")
               for i in range(NT)]

        def emit_once(stage_cb=None):
            # x^T tiles (host pre-transposed [C, T] fp16), one [128, T] tile
            # per 128-feature chunk.  Plain DMA; in staggered mode these are
            # stage 0 (they overlap the previous iteration's tail) and must
            # all issue from SP, which is idle there; otherwise alternate
            # between the two HWDGE rings (SP via nc.sync, ACT via
            # nc.scalar).
            xqc = []
            for c in range(NC):
                t_ = p_xq.tile([128, T], F16, tag="xq", name=f"xq{c}")
                eng = nc.sync if (stage_cb or c % 2 == 0) else nc.scalar
                eng.dma_start(out=t_[:],
                              in_=x_d[c * 128:(c + 1) * 128, :])
                xqc.append(t_)
            if stage_cb:
                stage_cb()

            # constants (tiny; re-done per pass so the hw loop stays honest)
            ones_t = p_const.tile([65, 64], F16, tag="ones")
            nc.vector.memset(ones_t[64:65, :], 1.0)
            trimask = p_const.tile([128, 2, 128], F16, tag="trimask")
            nc.gpsimd.dma_start(out=trimask[:], in_=mk_d[:])

            qtq = {}   # (f, qc) -> [128, 512] query quarter
            ytq = {}   # (f, qc) -> [128, 512] attention-out quarter

            def xq(c, n):
                return xqc[c][:, n * 512:(n + 1) * 512]

            # weights: one strided SWDGE DMA each (GpSimd queues — off the
            # HWDGE rings the transposes are using)
            w_sb = p_w.tile([128, NC, 3 * FPC], F16, tag="wqkv")
            nc.gpsimd.dma_start(
                out=w_sb[:], in_=wqkv_d.rearrange("(c p) f -> p c f", p=128))
            wp_sb = p_wp.tile([128, NF, C], F16, tag="wp")
            nc.gpsimd.dma_start(
                out=wp_sb[:], in_=wp_d.rearrange("(cf p) j -> p cf j", p=128))

            def q_group(n, f, isq):
                off = 0 if isq else FPC
                ps = ps_m.tile([128, 512], F32, tag="m1", name=f"qk{n}_{f}")
                for c in range(NC):
                    nc.tensor.matmul(
                        ps[:], w_sb[:, c, off + f * 128:off + (f + 1) * 128],
                        xq(c, n),
                        start=(c == 0), stop=(c == NC - 1))
                if isq:
                    dst = p_qtq.tile([128, 512], F16, tag="qt",
                                     name=f"qtq{f}_{n}")
                    qtq[(f, n)] = dst
                    nc.vector.tensor_copy(out=dst[:], in_=ps[:])
                else:
                    nc.vector.tensor_copy(
                        out=kt_[f][:, n * 512:(n + 1) * 512], in_=ps[:])

            def v_group(n, t):
                ps = ps_m.tile([128, FPC], F32, tag="m1", name=f"v{t}")
                for c in range(NC):
                    nc.tensor.matmul(
                        ps[:], xqc[c][:, t * 128:(t + 1) * 128],
                        w_sb[:, c, 2 * FPC:3 * FPC],
                        start=(c == 0), stop=(c == NC - 1))
                nc.vector.tensor_copy(
                    out=v65[t][:, :, 0:64],
                    in_=ps[:].rearrange("p (h d) -> p h d", h=HPC))
                nc.gpsimd.memset(v65[t][:, :, 64:65], 1.0)

            def qkv_groups(n):
                gs = []
                for f in range(NF):
                    gs.append(lambda f=f: q_group(n, f, True))
                for f in range(NF):
                    gs.append(lambda f=f: q_group(n, f, False))
                for t in range(4 * n, 4 * n + 4):
                    gs.append(lambda t=t: v_group(n, t))
                return gs

            pt0 = None
            if ablate == "noexp":
                pt0 = p_const.tile([128, 2, 512], F16, tag="pt0")
                nc.vector.memset(pt0[:], 0.002)

            def attention_hp(qc, hp):
                nk = 4 * qc + 4
                y_psA = ps_y.tile([65, 512], F32, tag="y")
                y_psB = ps_y.tile([65, 512], F32, tag="y")
                qtile = qtq[(hp, qc)]
                for kt in range(nk):
                    s_ps = ps_s.tile([128, 2, 512], F32, tag="s")
                    nc.tensor.matmul(
                        s_ps[:, 0, :],
                        kt_[hp][0:64, kt * 128:(kt + 1) * 128],
                        qtile[0:64, :],
                        start=True, stop=True, tile_position=(0, 0))
                    nc.tensor.matmul(
                        s_ps[:, 1, :],
                        kt_[hp][64:128, kt * 128:(kt + 1) * 128],
                        qtile[64:128, :],
                        start=True, stop=True, tile_position=(64, 0))
                    if ablate == "noexp":
                        nc.tensor.matmul(
                            y_psA[:], v65[kt][:, 2 * hp, :], pt0[:, 0, :],
                            start=(kt == 0), stop=(kt == nk - 1))
                        nc.tensor.matmul(
                            y_psB[:], v65[kt][:, 2 * hp + 1, :], pt0[:, 1, :],
                            start=(kt == 0), stop=(kt == nk - 1))
                        continue
                    pt = p_pt.tile([128, 2, 512], F16, tag="pt")
                    d = kt - 4 * qc
                    if kt < 4 * qc or d == 0:
                        # fully live, or diagonal with no dead prefix
                        nc.scalar.activation(
                            out=pt[:], in_=s_ps[:], func=EXP, scale=float(SCALE))
                    else:
                        dcol = 128 * d
                        nc.scalar.activation(
                            out=pt[:, :, dcol:512], in_=s_ps[:, :, dcol:512],
                            func=EXP, scale=float(SCALE))
                        nc.vector.memset(pt[:, :, 0:dcol], 0.0)
                    if d >= 0:
                        dcol = 128 * d
                        nc.vector.tensor_mul(
                            pt[:, :, dcol:dcol + 128], pt[:, :, dcol:dcol + 128],
                            trimask[:])
                    nc.tensor.matmul(
                        y_psA[:], v65[kt][:, 2 * hp, :], pt[:, 0, :],
                        start=(kt == 0), stop=(kt == nk - 1))
                    nc.tensor.matmul(
                        y_psB[:], v65[kt][:, 2 * hp + 1, :], pt[:, 1, :],
                        start=(kt == 0), stop=(kt == nk - 1))

                # softmax division; stage y psum to SBUF immediately so the
                # accumulator banks free for the next block
                ys = p_ys.tile([65, 2, 512], F32, tag="ys")
                nc.vector.tensor_copy(out=ys[:, 0, :], in_=y_psA[:])
                nc.vector.tensor_copy(out=ys[:, 1, :], in_=y_psB[:])

                ytile = p_ytq.tile([128, 512], F16, tag="yt",
                                   name=f"ytq{hp}_{qc}")
                ytq[(hp, qc)] = ytile

                # 1/denom via ACT ln -> exp(-x): DVE reciprocal is an
                # iterative-divide (~8 cyc/elem, ~4.3us per row); ln+exp are
                # two ACT ops covering BOTH heads' denom rows, sharing one
                # activation table set with the attention exps
                # (natural_log_exp).
                ln = p_rec.tile([65, 2, 512], F32, tag="ln")
                nc.scalar.activation(out=ln[64:65, :, :], in_=ys[64:65, :, :],
                                     func=LOG)
                rec = p_rec.tile([65, 2, 512], F16, tag="rec")
                nc.scalar.activation(out=rec[64:65, :, :], in_=ln[64:65, :, :],
                                     func=EXP, scale=-1.0)
                bcA = ps_m.tile([64, 512], F32, tag="m1")
                nc.tensor.matmul(
                    bcA[:], ones_t[64:65, :], rec[64:65, 0, :],
                    start=True, stop=True, tile_position=(64, 0))
                nc.vector.tensor_mul(ytile[0:64, :], ys[0:64, 0, :], bcA[:])

                bcB = ps_m.tile([64, 512], F32, tag="m1")
                nc.tensor.matmul(
                    bcB[:], ones_t[64:65, :], rec[64:65, 1, :],
                    start=True, stop=True, tile_position=(64, 0))
                ybt = p_ybt.tile([64, 512], F16, tag="ybt")
                nc.vector.tensor_mul(ybt[:], ys[0:64, 1, :], bcB[:])
                nc.gpsimd.dma_start(out=ytile[64:128, :], in_=ybt[:])

            def proj_t(qc, t, osb):
                tloc = (t - 4 * qc) * 128
                for nn in range(NN):
                    pj = ps_m.tile([128, npj], F32, tag="m1", name=f"pj{t}_{nn}")
                    for cf in range(NF):
                        nc.tensor.matmul(
                            pj[:],
                            ytq[(cf, qc)][:, tloc:tloc + 128],
                            wp_sb[:, cf, nn * npj:(nn + 1) * npj],
                            start=(cf == 0), stop=(cf == NF - 1))
                    nc.vector.tensor_copy(
                        out=osb[:, t - 4 * qc, nn * npj:(nn + 1) * npj], in_=pj[:])

            def proj_block(qc):
                osb = p_osb.tile([128, 4, C], F16, tag="osb", name=f"osb{qc}")
                for th in range(2):
                    for t in range(4 * qc + 2 * th, 4 * qc + 2 * th + 2):
                        proj_t(qc, t, osb)
                    nc.gpsimd.dma_start(
                        out=out_d[qc * 512 + th * 256:
                                  qc * 512 + (th + 1) * 256, :].rearrange(
                            "(tt p) j -> p tt j", p=128),
                        in_=osb[:, 2 * th:2 * th + 2, :])

            # software-pipelined emission: QKV for chunk qc+1 is emitted
            # BEFORE attention of chunk qc, and proj for chunk qc is emitted
            # one chunk late, so the scheduler always has independent PE work
            # (qkv early, proj late) to fill attention's dependency stalls —
            # in particular the last chunk's attention, which has no qkv
            # filler left.  Tile still tracks all dataflow deps.
            for g in qkv_groups(0):
                g()
            for qc in range(NQ):
                if qc + 1 < NQ:
                    for g in qkv_groups(qc + 1):
                        g()
                if parts == "qkv":
                    continue
                for hp in range(NF):
                    attention_hp(qc, hp)
                if parts == "attn":
                    continue
                if qc - 1 >= 0:
                    proj_block(qc - 1)
                if stage_cb and qc in (0, 2):
                    stage_cb()
            if parts == "full":
                proj_block(NQ - 1)
            if parts == "qkv":
                # timing-only variant: consume q/k/v so nothing is dead
                for f in range(NF):
                    nc.sync.dma_start(out=out_d[f * 128:(f + 1) * 128, 0:512],
                                      in_=qtq[(f, NQ - 1)][:])
            elif parts == "attn":
                for f in range(NF):
                    nc.sync.dma_start(out=out_d[f * 128:(f + 1) * 128, 0:512],
                                      in_=ytq[(f, NQ - 1)][:])

        if loop_n is None:
            emit_once()
        elif staggered:
            assert parts == "full" and ablate is None
            with tc.For_i(0, int(loop_n), 1, staggered_reset=True):
                emit_once(stage_cb=tc.stage_boundary)
        else:
            with tc.For_i(0, int(loop_n), 1):
                emit_once()

    # The greedy act-table-load pass picks a table set per activation; with
    # Exp resolving to "exp_and_others" and Ln to
    # "natural_log_exp_and_others" it would thrash sets (~2.7us per reload,
    # 65 reloads).  Keep the original set order (act_func_set_id is an index
    # into act_info.json) but hide Exp from every other set, so both Exp and
    # Ln resolve to the one set containing both -> exactly one load.
    import concourse.bacc as _bacc_mod
    _orig_tables = _bacc_mod.get_activation_tables

    def _tables_ln_exp_only(arch):
        tabs = _orig_tables(arch)
        both = "natural_log_exp_and_others"
        if both in tabs:
            for name, fns in tabs.items():
                if name != both:
                    fns.discard(EXP)
        return tabs

    _bacc_mod.get_activation_tables = _tables_ln_exp_only
    try:
        nc.finalize()
    finally:
        _bacc_mod.get_activation_tables = _orig_tables
    return nc


def _make_masks():
    kk = np.arange(128)[:, None]
    jj = np.arange(128)[None, :]
    m = (jj >= kk).astype(NPF16)          # [k, q] lower-left live (q >= k)
    return np.ascontiguousarray(np.broadcast_to(m[:, None, :], (128, 2, 128)))


def make_in_maps(x, W_qkv, W_proj):
    """Host-side sharding of full inputs into per-core input maps (fp16)."""
    x = np.asarray(x)
    W_qkv = np.asarray(W_qkv)
    W_proj = np.asarray(W_proj)
    xh = [np.ascontiguousarray(x[b].T, dtype=NPF16) for b in range(B)]
    masks = _make_masks()
    wqkv = [np.concatenate(
        [W_qkv[:, s * C + g * FPC:s * C + (g + 1) * FPC] for s in range(3)],
        axis=1).astype(NPF16) for g in range(GROUPS)]
    wps = [np.ascontiguousarray(W_proj[g * FPC:(g + 1) * FPC, :], dtype=NPF16)
           for g in range(GROUPS)]
    in_maps = []
    for core in range(NCORES):
        b, g = core // GROUPS, core % GROUPS
        in_maps.append({
            "xT": xh[b],
            "wqkv": wqkv[g],
            "wp": wps[g],
            "trimask2": masks,
        })
    return in_maps


_CACHE = {}


def _get_nc():
    if "nc" not in _CACHE:
        _CACHE["nc"] = build_nc()
    return _CACHE["nc"]


def run_cores(in_maps):
    res = run_bass_kernel_spmd(_get_nc(), in_maps, list(range(NCORES)))
    return res.results


def kernel(x, W_qkv, W_proj):
    results = run_cores(make_in_maps(x, W_qkv, W_proj))
    out = np.empty((B, T, C), dtype=np.float32)
    for b in range(B):
        out[b] = results[GROUPS * b]["out"].astype(np.float32)
        for g in range(1, GROUPS):
            out[b] += results[GROUPS * b + g]["out"].astype(np.float32)
    return out


# revision 50
# speedup vs baseline: 1365.4248x; 1.0157x over previous
"""Causal self-attention TRN2 kernel (v2).

Full module: x[4,2048,1024] @ W_qkv[1024,3072] -> heads(16, d=64) causal attn
-> @ W_proj[1024,1024].

Sharding: 8 cores = 4 batches x 2 head-groups (8 heads each), tensor-parallel
over heads. Each core computes q/k/v for its 8 heads, causal attention, and a
partial projection (row-sharded W_proj). The two partials per batch are summed
on the host (no on-device collectives).

v2 changes vs the f32r baseline (517 -> ~400 us/pass measured via the
hardware-loop differencing in test.py):
  - All streamed tensors fp16 (xT, packed W_qkv, W_proj, K/V/Q tiles, P~, y,
    output); PSUM accumulation stays fp32.  Halves HBM bytes, enables DVE 2x
    packed modes and PE fast-weight-load.  Measured end-to-end rel err ~5e-4
    (tolerance 2e-2).
  - DMA traffic spread over all three parallel issue paths: xT tiles on the
    two HWDGE rings (nc.sync + nc.scalar), weights / output / small copies
    on the GpSimd SWDGE queues; weights coalesced to ONE strided DMA each.
  - Causal masking: the fully-masked column range of diagonal score tiles is
    never exp'd (live-range ACT + DVE memset) instead of exp-then-zero-DMA
    from an HBM zeros tensor; the triangular block of both heads is masked
    by ONE strided tensor_mul against a duplicated [128,2,128] mask.
  - 1/denom via ACT ln->exp(-x) instead of DVE reciprocal (iterative divide,
    ~8 cyc/elem = ~4.3us per row); both heads' denom rows in one ln and one
    exp.  A scoped patch keeps Exp+Ln in the single
    natural_log_exp_and_others activation-table set (no 2.7us set thrash).
  - Software-pipelined emission: QKV of chunk qc+1 before attention of qc,
    proj of qc one chunk late, so the Tile scheduler always has independent
    PE work to fill attention's cross-engine dependency stalls.
  - build_nc(loop_n=R) wraps the whole per-pass computation in a tc.For_i
    hardware loop (same NEFF size for any R) for wall-clock-differenced
    device timing; staggered=True variant exists but measured slower.

Per-core program, per 512-token q-chunk qc (as baseline):
  QKV:   qT quarters [128f, 512t] (head-pair-major features), kT chunks
         [128f, T], V tiles [128t, 8 heads, 64 V + 1 ones col]. Matmuls fp16
         in, fp32 PSUM out.
  attn:  scores^T [k,q] per 128-k-tile: head pair row-packed into one PE pass
         (K=64 halves at tile_position (0,0)/(64,0), concurrent on HW)
         writing one 2-bank PSUM tile; ONE exp per k-tile on ACT covers both
         heads (scale=1/8 folded in); y^T += [V|1]^T @ P~ accumulates in
         PSUM with row 64 = softmax denominators; divide via ln/exp + K=1
         matmul partition-broadcast of 1/denom.
  proj:  out[t,:] partial = yT^T @ wp per q-chunk, stored fp16.
"""

import numpy as np
from contextlib import ExitStack

import concourse.bass as bass
import concourse.tile as tile
from concourse import mybir, bacc
from concourse.bass_utils import run_bass_kernel_spmd

F32 = mybir.dt.float32
F16 = mybir.dt.float16
EXP = mybir.ActivationFunctionType.Exp
LOG = mybir.ActivationFunctionType.Ln

B, T, C, H, D = 4, 2048, 1024, 16, 64
NCORES = 8
GROUPS = 2            # head groups (tensor-parallel dimension)
HPC = H // GROUPS     # heads per core = 8
FPC = HPC * D         # features per core = 512
SCALE = 1.0 / np.sqrt(D)

NPF16 = np.float16  # host-side dtype matching the F16 device tensors


def build_nc(T=T, C=C, HPC=HPC, loop_n=None, parts="full", tune=None,
             ablate=None, staggered=False):
    tu = {"pt": 5, "ys": 2, "rec": 1, "osb": 2, "ybt": 1, "qtq": None,
          "ytq": 12}
    if tune:
        tu.update(tune)
    FPC = HPC * D
    NC = C // 128     # contraction chunks over C
    NT = T // 128     # token tiles (also k-tiles)
    NQ = T // 512     # query chunks (= merged pipeline blocks)
    NF = FPC // 128   # feature tiles = head pairs
    NN = max(C // 512, 1)  # proj output column chunks
    npj = min(512, C)

    nc = bacc.Bacc("TRN2", debug=False)
    x_d = nc.dram_tensor("xT", [C, T], F16, kind="ExternalInput").ap()
    wqkv_d = nc.dram_tensor("wqkv", [C, 3 * FPC], F16, kind="ExternalInput").ap()
    wp_d = nc.dram_tensor("wp", [FPC, C], F16, kind="ExternalInput").ap()
    mk_d = nc.dram_tensor("trimask2", [128, 2, 128], F16, kind="ExternalInput").ap()
    out_d = nc.dram_tensor("out", [T, C], F16, kind="ExternalOutput").ap()

    with tile.TileContext(nc) as tc, ExitStack() as ctx:
        p_kt = ctx.enter_context(tc.tile_pool(name="p_kt", bufs=NF))
        p_v65 = ctx.enter_context(tc.tile_pool(name="p_v65", bufs=NT))
        p_const = ctx.enter_context(tc.tile_pool(name="p_const", bufs=1))
        p_w = ctx.enter_context(tc.tile_pool(name="p_w", bufs=1))
        p_xq = ctx.enter_context(tc.tile_pool(name="p_xq", bufs=NC))
        p_qtq = ctx.enter_context(
            tc.tile_pool(name="p_qtq", bufs=tu["qtq"] or 2 * NF))
        p_ytq = ctx.enter_context(
            tc.tile_pool(name="p_ytq", bufs=tu["ytq"] or 2 * NF))
        p_pt = ctx.enter_context(tc.tile_pool(name="p_pt", bufs=tu["pt"]))
        p_rec = ctx.enter_context(tc.tile_pool(name="p_rec", bufs=tu["rec"]))
        p_ys = ctx.enter_context(tc.tile_pool(name="p_ys", bufs=tu["ys"]))
        p_ybt = ctx.enter_context(tc.tile_pool(name="p_ybt", bufs=tu["ybt"]))
        p_wp = ctx.enter_context(tc.tile_pool(name="p_wp", bufs=1))
        p_osb = ctx.enter_context(tc.tile_pool(name="p_osb", bufs=tu["osb"]))
        # one shared PSUM budget, 8 banks: s 2x2 + y 2 + misc 2
        ps_s = ctx.enter_context(tc.tile_pool(name="ps_s", bufs=2, space="PSUM"))
        ps_y = ctx.enter_context(tc.tile_pool(name="ps_y", bufs=2, space="PSUM"))
        ps_m = ctx.enter_context(tc.tile_pool(name="ps_m", bufs=2, space="PSUM"))

        kt_ = [p_kt.tile([128, T], F16, tag="kt", name=f"kt{i}") for i in range(NF)]
        v65 = [p_v65.tile([128, HPC, 65], F16, tag="v65", name=f"v65_{i}")
               for i in range(NT)]

        def emit_once(stage_cb=None):
            # x^T tiles (host pre-transposed [C, T] fp16), one [128, T] tile
            # per 128-feature chunk.  Plain DMA; in staggered mode these are
            # stage 0 (they overlap the previous iteration's tail) and must
            # all issue from SP, which is idle there; otherwise alternate
            # between the two HWDGE rings (SP via nc.sync, ACT via
            # nc.scalar).
            xqc = []
            for c in range(NC):
                t_ = p_xq.tile([128, T], F16, tag="xq", name=f"xq{c}")
                eng = nc.sync if (stage_cb or c % 2 == 0) else nc.scalar
                eng.dma_start(out=t_[:],
                              in_=x_d[c * 128:(c + 1) * 128, :])
                xqc.append(t_)
            if stage_cb:
                stage_cb()

            # constants (tiny; re-done per pass so the hw loop stays honest)
            ones_t = p_const.tile([65, 64], F16, tag="ones")
            nc.vector.memset(ones_t[64:65, :], 1.0)
            trimask = p_const.tile([128, 2, 128], F16, tag="trimask")
            nc.gpsimd.dma_start(out=trimask[:], in_=mk_d[:])

            qtq = {}   # (f, qc) -> [128, 512] query quarter
            ytq = {}   # (f, qc) -> [128, 512] attention-out quarter

            def xq(c, n):
                return xqc[c][:, n * 512:(n + 1) * 512]

            # weights: one strided SWDGE DMA each (GpSimd queues — off the
            # HWDGE rings the transposes are using)
            w_sb = p_w.tile([128, NC, 3 * FPC], F16, tag="wqkv")
            nc.gpsimd.dma_start(
                out=w_sb[:], in_=wqkv_d.rearrange("(c p) f -> p c f", p=128))
            wp_sb = p_wp.tile([128, NF, C], F16, tag="wp")
            nc.gpsimd.dma_start(
                out=wp_sb[:], in_=wp_d.rearrange("(cf p) j -> p cf j", p=128))

            def q_group(n, f, isq):
                off = 0 if isq else FPC
                ps = ps_m.tile([128, 512], F32, tag="m1", name=f"qk{n}_{f}")
                for c in range(NC):
                    nc.tensor.matmul(
                        ps[:], w_sb[:, c, off + f * 128:off + (f + 1) * 128],
                        xq(c, n),
                        start=(c == 0), stop=(c == NC - 1))
                if isq:
                    dst = p_qtq.tile([128, 512], F16, tag="qt",
                                     name=f"qtq{f}_{n}")
                    qtq[(f, n)] = dst
                    nc.vector.tensor_copy(out=dst[:], in_=ps[:])
                else:
                    nc.vector.tensor_copy(
                        out=kt_[f][:, n * 512:(n + 1) * 512], in_=ps[:])

            def v_group(n, t):
                ps = ps_m.tile([128, FPC], F32, tag="m1", name=f"v{t}")
                for c in range(NC):
                    nc.tensor.matmul(
                        ps[:], xqc[c][:, t * 128:(t + 1) * 128],
                        w_sb[:, c, 2 * FPC:3 * FPC],
                        start=(c == 0), stop=(c == NC - 1))
                nc.vector.tensor_copy(
                    out=v65[t][:, :, 0:64],
                    in_=ps[:].rearrange("p (h d) -> p h d", h=HPC))
                nc.gpsimd.memset(v65[t][:, :, 64:65], 1.0)

            def qkv_groups(n):
                gs = []
                for f in range(NF):
                    gs.append(lambda f=f: q_group(n, f, True))
                for f in range(NF):
                    gs.append(lambda f=f: q_group(n, f, False))
                for t in range(4 * n, 4 * n + 4):
                    gs.append(lambda t=t: v_group(n, t))
                return gs

            pt0 = None
            if ablate == "noexp":
                pt0 = p_const.tile([128, 2, 512], F16, tag="pt0")
                nc.vector.memset(pt0[:], 0.002)

            def attention_hp(qc, hp):
                nk = 4 * qc + 4
                y_psA = ps_y.tile([65, 512], F32, tag="y")
                y_psB = ps_y.tile([65, 512], F32, tag="y")
                qtile = qtq[(hp, qc)]
                for kt in range(nk):
                    # diagonal k-tiles only touch q >= lo: the masked prefix
                    # [0:lo) gets no contribution from this tile, so score,
                    # exp and AV all stream the live q-range only (and the
                    # dead-region memset disappears)
                    d = kt - 4 * qc
                    lo = 128 * d if d > 0 else 0
                    s_ps = ps_s.tile([128, 2, 512], F32, tag="s")
                    nc.tensor.matmul(
                        s_ps[:, 0, lo:512],
                        kt_[hp][0:64, kt * 128:(kt + 1) * 128],
                        qtile[0:64, lo:512],
                        start=True, stop=True, tile_position=(0, 0))
                    nc.tensor.matmul(
                        s_ps[:, 1, lo:512],
                        kt_[hp][64:128, kt * 128:(kt + 1) * 128],
                        qtile[64:128, lo:512],
                        start=True, stop=True, tile_position=(64, 0))
                    if ablate == "noexp":
                        nc.tensor.matmul(
                            y_psA[:], v65[kt][:, 2 * hp, :], pt0[:, 0, :],
                            start=(kt == 0), stop=(kt == nk - 1))
                        nc.tensor.matmul(
                            y_psB[:], v65[kt][:, 2 * hp + 1, :], pt0[:, 1, :],
                            start=(kt == 0), stop=(kt == nk - 1))
                        continue
                    pt = p_pt.tile([128, 2, 512], F16, tag="pt")
                    nc.scalar.activation(
                        out=pt[:, :, lo:512], in_=s_ps[:, :, lo:512],
                        func=EXP, scale=float(SCALE))
                    if d >= 0:
                        dcol = 128 * d
                        nc.vector.tensor_mul(
                            pt[:, :, dcol:dcol + 128], pt[:, :, dcol:dcol + 128],
                            trimask[:])
                    nc.tensor.matmul(
                        y_psA[:, lo:512], v65[kt][:, 2 * hp, :],
                        pt[:, 0, lo:512],
                        start=(kt == 0), stop=(kt == nk - 1))
                    nc.tensor.matmul(
                        y_psB[:, lo:512], v65[kt][:, 2 * hp + 1, :],
                        pt[:, 1, lo:512],
                        start=(kt == 0), stop=(kt == nk - 1))

                # softmax division; stage y psum to SBUF immediately so the
                # accumulator banks free for the next block
                ys = p_ys.tile([65, 2, 512], F32, tag="ys")
                nc.vector.tensor_copy(out=ys[:, 0, :], in_=y_psA[:])
                nc.vector.tensor_copy(out=ys[:, 1, :], in_=y_psB[:])

                ytile = p_ytq.tile([128, 512], F16, tag="yt",
                                   name=f"ytq{hp}_{qc}")
                ytq[(hp, qc)] = ytile

                # 1/denom via ACT ln -> exp(-x): DVE reciprocal is an
                # iterative-divide (~8 cyc/elem, ~4.3us per row); ln+exp are
                # two ACT ops covering BOTH heads' denom rows, sharing one
                # activation table set with the attention exps
                # (natural_log_exp).
                ln = p_rec.tile([65, 2, 512], F32, tag="ln")
                nc.scalar.activation(out=ln[64:65, :, :], in_=ys[64:65, :, :],
                                     func=LOG)
                rec = p_rec.tile([65, 2, 512], F16, tag="rec")
                nc.scalar.activation(out=rec[64:65, :, :], in_=ln[64:65, :, :],
                                     func=EXP, scale=-1.0)
                bcA = ps_m.tile([64, 512], F32, tag="m1")
                nc.tensor.matmul(
                    bcA[:], ones_t[64:65, :], rec[64:65, 0, :],
                    start=True, stop=True, tile_position=(64, 0))
                nc.vector.tensor_mul(ytile[0:64, :], ys[0:64, 0, :], bcA[:])

                bcB = ps_m.tile([64, 512], F32, tag="m1")
                nc.tensor.matmul(
                    bcB[:], ones_t[64:65, :], rec[64:65, 1, :],
                    start=True, stop=True, tile_position=(64, 0))
                ybt = p_ybt.tile([64, 512], F16, tag="ybt")
                nc.vector.tensor_mul(ybt[:], ys[0:64, 1, :], bcB[:])
                nc.gpsimd.dma_start(out=ytile[64:128, :], in_=ybt[:])

            def proj_t(qc, t, osb):
                tloc = (t - 4 * qc) * 128
                for nn in range(NN):
                    pj = ps_m.tile([128, npj], F32, tag="m1", name=f"pj{t}_{nn}")
                    for cf in range(NF):
                        nc.tensor.matmul(
                            pj[:],
                            ytq[(cf, qc)][:, tloc:tloc + 128],
                            wp_sb[:, cf, nn * npj:(nn + 1) * npj],
                            start=(cf == 0), stop=(cf == NF - 1))
                    nc.vector.tensor_copy(
                        out=osb[:, t - 4 * qc, nn * npj:(nn + 1) * npj], in_=pj[:])

            def proj_block(qc):
                osb = p_osb.tile([128, 4, C], F16, tag="osb", name=f"osb{qc}")
                for th in range(2):
                    for t in range(4 * qc + 2 * th, 4 * qc + 2 * th + 2):
                        proj_t(qc, t, osb)
                    nc.gpsimd.dma_start(
                        out=out_d[qc * 512 + th * 256:
                                  qc * 512 + (th + 1) * 256, :].rearrange(
                            "(tt p) j -> p tt j", p=128),
                        in_=osb[:, 2 * th:2 * th + 2, :])

            # software-pipelined emission: QKV for chunk qc+1 is emitted
            # BEFORE attention of chunk qc, and proj for chunk qc is emitted
            # one chunk late, so the scheduler always has independent PE work
            # (qkv early, proj late) to fill attention's dependency stalls —
            # in particular the last chunk's attention, which has no qkv
            # filler left.  Tile still tracks all dataflow deps.
            for g in qkv_groups(0):
                g()
            for qc in range(NQ):
                if qc + 1 < NQ:
                    for g in qkv_groups(qc + 1):
                        g()
                if parts == "qkv":
                    continue
                for hp in range(NF):
                    attention_hp(qc, hp)
                if parts == "attn":
                    continue
                if qc - 1 >= 0:
                    proj_block(qc - 1)
                if stage_cb and qc in (0, 2):
                    stage_cb()
            if parts == "full":
                proj_block(NQ - 1)
            if parts == "qkv":
                # timing-only variant: consume q/k/v so nothing is dead
                for f in range(NF):
                    nc.sync.dma_start(out=out_d[f * 128:(f + 1) * 128, 0:512],
                                      in_=qtq[(f, NQ - 1)][:])
            elif parts == "attn":
                for f in range(NF):
                    nc.sync.dma_start(out=out_d[f * 128:(f + 1) * 128, 0:512],
                                      in_=ytq[(f, NQ - 1)][:])

        if loop_n is None:
            emit_once()
        elif staggered:
            assert parts == "full" and ablate is None
            with tc.For_i(0, int(loop_n), 1, staggered_reset=True):
                emit_once(stage_cb=tc.stage_boundary)
        else:
            with tc.For_i(0, int(loop_n), 1):
                emit_once()

    # The greedy act-table-load pass picks a table set per activation; with
    # Exp resolving to "exp_and_others" and Ln to
    # "natural_log_exp_and_others" it would thrash sets (~2.7us per reload,
    # 65 reloads).  Keep the original set order (act_func_set_id is an index
    # into act_info.json) but hide Exp from every other set, so both Exp and
    # Ln resolve to the one set containing both -> exactly one load.
    import concourse.bacc as _bacc_mod
    _orig_tables = _bacc_mod.get_activation_tables

    def _tables_ln_exp_only(arch):
        tabs = _orig_tables(arch)
        both = "natural_log_exp_and_others"
        if both in tabs:
            for name, fns in tabs.items():
                if name != both:
                    fns.discard(EXP)
        return tabs

    _bacc_mod.get_activation_tables = _tables_ln_exp_only
    try:
        nc.finalize()
    finally:
        _bacc_mod.get_activation_tables = _orig_tables
    return nc


def _make_masks():
    kk = np.arange(128)[:, None]
    jj = np.arange(128)[None, :]
    m = (jj >= kk).astype(NPF16)          # [k, q] lower-left live (q >= k)
    return np.ascontiguousarray(np.broadcast_to(m[:, None, :], (128, 2, 128)))


def make_in_maps(x, W_qkv, W_proj):
    """Host-side sharding of full inputs into per-core input maps (fp16)."""
    x = np.asarray(x)
    W_qkv = np.asarray(W_qkv)
    W_proj = np.asarray(W_proj)
    xh = [np.ascontiguousarray(x[b].T, dtype=NPF16) for b in range(B)]
    masks = _make_masks()
    wqkv = [np.concatenate(
        [W_qkv[:, s * C + g * FPC:s * C + (g + 1) * FPC] for s in range(3)],
        axis=1).astype(NPF16) for g in range(GROUPS)]
    wps = [np.ascontiguousarray(W_proj[g * FPC:(g + 1) * FPC, :], dtype=NPF16)
           for g in range(GROUPS)]
    in_maps = []
    for core in range(NCORES):
        b, g = core // GROUPS, core % GROUPS
        in_maps.append({
            "xT": xh[b],
            "wqkv": wqkv[g],
            "wp": wps[g],
            "trimask2": masks,
        })
    return in_maps


_CACHE = {}


def _get_nc():
    if "nc" not in _CACHE:
        _CACHE["nc"] = build_nc()
    return _CACHE["nc"]


def run_cores(in_maps):
    res = run_bass_kernel_spmd(_get_nc(), in_maps, list(range(NCORES)))
    return res.results


def kernel(x, W_qkv, W_proj):
    results = run_cores(make_in_maps(x, W_qkv, W_proj))
    out = np.empty((B, T, C), dtype=np.float32)
    for b in range(B):
        out[b] = results[GROUPS * b]["out"].astype(np.float32)
        for g in range(1, GROUPS):
            out[b] += results[GROUPS * b + g]["out"].astype(np.float32)
    return out
